# revision 1
# baseline (speedup 1.0000x reference)
"""CycleMLP 1w1a (binary cycle-shift conv + 1x1 GEMM) for 8 Trainium2 cores.

  out[b,o,h,w] = sum_c sign(weight)[o,c] * sign(x)[b,c,h,w+off(c)] + bias[o]
  off(c) = (c+3) % 7 - 3, zero-padded outside [0, W)

Sharding: data-parallel over batch B=64 -> 8 batches/core; weight/bias
replicated (prepped host-side: sign, channel permutation, bf16 lhsT layout).

Per-core kernel:
  - channels permuted by residue c % 7 so each shift-group is a contiguous
    partition range; the weight's contraction dim is permuted identically.
  - x is DMA'd with the flat h*W+w index shifted by the group's offset d
    (contiguous 4KB-per-channel runs).  Columns where w+d leaves [0, W)
    receive leaked neighbor-row data and are zeroed via a bf16 mask multiply.
  - sign() on ScalarE f32 -> bf16 (+-1 exact in bf16; fp32 PSUM accumulation
    of +-1 terms is exact, so results match the fp32 reference bitwise).
  - GEMM on TensorE: 3 K-chunks x 3 M-chunks x 512-col N-tiles, PSUM
    accumulation over K, bias fused into the DVE eviction.
"""

import sys

for p in ("/opt/trn_rl_repo", "/root/.axon_site/_ro/trn_rl_repo"):
    if p not in sys.path:
        sys.path.append(p)

import numpy as np

B = 64
C = 384
H = W = 32
HW = H * W
KW = 7
NK = 3  # contraction chunks of 128
NM = 3  # output-channel chunks of 128
NTILE = 512
N_CORES = 8
SB = B // N_CORES  # batches per core
BG = 2  # batches per pipeline group

_CACHE = {}


def _off(c):
    return (c + 3) % KW - KW // 2


def _chunk_pieces(k):
    """DMA pieces for chunk k (channels [128k, 128k+128), natural order).

    d(c) = (c+3)%7-3 increments by +1 between consecutive channels except
    at c % 7 == 3 -> 4 (where it wraps 3 -> -3).  So between run starts
    (c % 7 == 4) the per-channel source offset c*HW + d(c) advances by a
    constant HW+1, and run starts advance by 7*HW.  Pieces:
      ('lat', p0, len, c_start)           lattice [HW+1, len]
      ('runs', p0, nruns, c_start)        lattice [[7HW, nruns], [HW+1, 7]]
    """
    c0, c1 = 128 * k, 128 * k + 128
    rs0 = c0 + ((4 - c0) % 7)
    pieces = []
    if rs0 > c0:
        pieces.append(("lat", 0, rs0 - c0, c0))
    n = (c1 - rs0) // 7
    if n > 0:
        pieces.append(("runs", rs0 - c0, n, rs0))
    tail = rs0 + 7 * n
    if tail < c1:
        pieces.append(("lat", tail - c0, c1 - tail, tail))
    return pieces


def _prep_weights(weight, bias):
    import ml_dtypes

    wb = np.sign(weight.astype(np.float32))  # [O, C]
    lhsT = np.ascontiguousarray(wb.T)  # [C, O]
    wt = np.ascontiguousarray(lhsT.reshape(NK, 128, C).transpose(1, 0, 2)).astype(
        ml_dtypes.bfloat16
    )  # [128, NK, C]
    bias_sb = np.ascontiguousarray(bias.astype(np.float32).reshape(NM, 128).T)

    mask = np.ones((128, NK, W), dtype=np.float32)
    for k in range(NK):
        for p in range(128):
            d = _off(128 * k + p)
            if d > 0:
                mask[p, k, W - d : W] = 0.0
            elif d < 0:
                mask[p, k, 0 : -d] = 0.0
    mask = mask.astype(ml_dtypes.bfloat16)
    return wt, bias_sb, mask


def _legalize_waits(nc, max_waits=1):
    """Walrus for this toolchain accepts at most one sem wait per
    instruction.  Split instructions carrying more into preceding
    same-engine NoOps (engine streams are in-order, so the split is
    semantically identical to the combined wait)."""
    import concourse.mybir as mybir

    fn = nc.m.functions[0]
    ctr = 0
    for blk in fn.blocks:
        out = []
        changed = False
        for inst in blk.instructions:
            si = inst.sync_info
            waits = list(si.on_wait) if si is not None and si.on_wait else []
            if len(waits) > max_waits and str(inst.engine) != "EngineType.Unassigned":
                keep = waits[-max_waits:]
                extra = waits[:-max_waits]
                for j in range(0, len(extra), max_waits):
                    nop = mybir.InstNoOp(name=f"I-waitsplit-{ctr}")
                    ctr += 1
                    nop.engine = inst.engine
                    nop.sync_info = mybir.SyncInfo(
                        on_wait=extra[j : j + max_waits], on_update=[]
                    )
                    out.append(nop)
                si.on_wait = keep
                changed = True
            out.append(inst)
        if changed:
            blk.instructions = out
    return ctr


def _build(raw_bufs=4, psum_bufs=6, ost_bufs=4, g_bufs=2, legalize=True):
    import concourse.bass as bass
    import concourse.mybir as mybir
    import concourse.tile as tile
    from concourse.ap import AP

    nc = bass.Bass()
    x_d = nc.declare_dram_parameter("x", [SB, C, HW], mybir.dt.float32, isOutput=False)
    wt_d = nc.declare_dram_parameter("wt", [128, NK, C], mybir.dt.bfloat16, isOutput=False)
    bias_d = nc.declare_dram_parameter("bias", [128, NM], mybir.dt.float32, isOutput=False)
    mask_d = nc.declare_dram_parameter("mask", [128, NK, W], mybir.dt.bfloat16, isOutput=False)
    out_d = nc.declare_dram_parameter("out", [SB, C, HW], mybir.dt.float32, isOutput=True)

    with tile.TileContext(nc) as tc:
        with (
            tc.tile_pool(name="const", bufs=1) as const_pool,
            tc.tile_pool(name="raw", bufs=raw_bufs) as raw_pool,
            tc.tile_pool(name="g", bufs=g_bufs) as g_pool,
            tc.tile_pool(name="ost", bufs=ost_bufs) as ost_pool,
            tc.tile_pool(name="ps", bufs=psum_bufs, space="PSUM") as ps_pool,
        ):
            wt = const_pool.tile([128, NK, C], mybir.dt.bfloat16)
            bias_sb = const_pool.tile([128, NM], mybir.dt.float32)
            mask_sb = const_pool.tile([128, NK, W], mybir.dt.bfloat16)
            nc.sync.dma_start(wt[:], wt_d[:])
            nc.sync.dma_start(bias_sb[:], bias_d[:])
            nc.sync.dma_start(mask_sb[:], mask_d[:])

            for b in range(SB):
                g = []
                for k in range(NK):
                    # dense [128, HW] tiles: the HWDGE engine-split fans a
                    # DMA across all 16 SDMA engines only when the SBUF-side
                    # AP is dense 2D (partition stride == row size);
                    # strided tiles serialize onto one engine.
                    raw = raw_pool.tile([128, HW], mybir.dt.float32, tag="raw")
                    for piece in _chunk_pieces(k):
                        kind, p0, n, cs = piece
                        base = b * C * HW + cs * HW + _off(cs)
                        if kind == "lat":
                            src = AP(
                                tensor=x_d,
                                offset=base,
                                ap=[[HW + 1, n], [1, HW]],
                            )
                            dst = raw[p0 : p0 + n, :]
                        else:
                            src = AP(
                                tensor=x_d,
                                offset=base,
                                ap=[[7 * HW, n], [HW + 1, 7], [1, HW]],
                            )
                            dst = raw[p0 : p0 + 7 * n, :]
                        nc.sync.dma_start(dst, src)
                    gk = g_pool.tile([128, HW], mybir.dt.bfloat16, tag=f"g{k}")
                    nc.scalar.sign(gk[:], raw[:])
                    v = gk.rearrange("p (h w) -> p h w", w=W)
                    mk = mask_sb[:, k : k + 1, :].broadcast_to([128, H, W])
                    nc.vector.tensor_mul(v, v, mk)
                    g.append(gk)

                for m in range(NM):
                    ost = ost_pool.tile([128, HW], mybir.dt.float32, tag="ost")
                    for n in range(HW // NTILE):
                        ps = ps_pool.tile([128, NTILE], mybir.dt.float32, tag="ps")
                        for k in range(NK):
                            nc.tensor.matmul(
                                ps[:],
                                wt[:, k, m * 128 : (m + 1) * 128],
                                g[k][:, n * NTILE : (n + 1) * NTILE],
                                start=(k == 0),
                                stop=(k == NK - 1),
                            )
                        nc.vector.tensor_scalar_add(
                            ost[:, n * NTILE : (n + 1) * NTILE],
                            ps[:],
                            bias_sb[:, m : m + 1],
                        )
                    # stores go out on the ACT HWDGE ring to split sequencer
                    # issue load between the two rings
                    nc.scalar.dma_start(
                        out_d[b, m * 128 : (m + 1) * 128, :], ost[:]
                    )
    if legalize:
        _legalize_waits(nc)
    return nc


def _ensure_ntff_hook():
    """Register the axon NTFF profiling hook if the image's antenv lacks it."""
    import types

    try:
        from antenv.axon_hooks import get_axon_ntff_profile_hook  # noqa: F401

        return
    except ImportError:
        pass
    hook = None
    try:
        from trn_agent_boot.trn_boot import _ntff_profile_via_ctypes

        hook = _ntff_profile_via_ctypes("/opt/axon/libaxon_pjrt.so")
    except Exception:
        pass
    mod = types.ModuleType("antenv.axon_hooks")
    mod._hook = hook
    mod.get_axon_ntff_profile_hook = lambda: mod._hook
    mod.set_axon_ntff_profile_hook = lambda h: setattr(mod, "_hook", h)
    sys.modules["antenv.axon_hooks"] = mod
    try:
        import antenv

        antenv.axon_hooks = mod
    except Exception:
        pass


def run(x, weight, bias, trace=False):
    """Returns (out [B,C,H,W] f32, exec_time_ns or None)."""
    import concourse.bass_utils as bu
    from concourse.bass_utils import run_bass_kernel_spmd

    if trace:
        _ensure_ntff_hook()
        # zero-egress container: don't try to copy trace artifacts to a bucket
        bu.upload_artifacts = lambda tmpdir: tmpdir

    if "nc" not in _CACHE:
        _CACHE["nc"] = _build()
    nc = _CACHE["nc"]

    wt, bias_sb, mask = _prep_weights(weight, bias)
    x = np.ascontiguousarray(x.astype(np.float32, copy=False)).reshape(B, C, HW)
    in_maps = [
        {
            "x": x[i * SB : (i + 1) * SB],
            "wt": wt,
            "bias": bias_sb,
            "mask": mask,
        }
        for i in range(N_CORES)
    ]
    res = run_bass_kernel_spmd(
        nc, in_maps, core_ids=list(range(N_CORES)), trace=trace
    )
    out = np.concatenate([res.results[i]["out"] for i in range(N_CORES)], axis=0)
    return out.reshape(B, C, H, W).astype(np.float32, copy=False), res.exec_time_ns


def kernel(x, weight, bias):
    out, _ = run(x, weight, bias, trace=False)
    return out



# revision 8
# speedup vs baseline: 2.2061x; 2.2061x over previous
"""CycleMLP 1w1a (binary cycle-shift conv + 1x1 GEMM) for 8 Trainium2 cores.

  out[b,o,h,w] = sum_c sign(weight)[o,c] * sign(x)[b,c,h,w+off(c)] + bias[o]
  off(c) = (c+3) % 7 - 3, zero-padded outside [0, W)

Sharding: data-parallel over batch B=64 -> 8 batches/core; weight/bias
replicated.

Key layout tricks (all host-side, mathematically identity):
  - channels permuted by residue c % 7 so every shift-group is a contiguous
    channel range sharing ONE offset d; the weight's contraction dim is
    permuted identically.  Each input load is then a fully CONTIGUOUS 2D
    DMA with the shift folded into the base address (dense 2D transfers
    fan across all 16 SDMA engines; lattice APs do not).
  - per-core layout [G, C, BG*HW] (channel-major over a half-batch group)
    so one segment load covers all batches of the group: ~9 big loads and
    3 stores per group instead of ~100 small DMAs.
  - bf16 x (sign-preserving) and bf16 out (exact integer sums <= 384 round
    with ulp 0.125 -> rel err ~2e-3) halve HBM traffic.

Per-core kernel: per group, per 128-channel chunk: 3 contiguous loads ->
sign (ScalarE) -> boundary-column memsets (GpSimd; replaces mask mul) ->
GEMM 3m x 8n x 3k with PSUM K-accumulation -> bias-add eviction to bf16
(DVE) -> one store per m-block.
"""

import sys

for p in ("/opt/trn_rl_repo", "/root/.axon_site/_ro/trn_rl_repo"):
    if p not in sys.path:
        sys.path.append(p)

import numpy as np

B = 64
C = 384
H = W = 32
HW = H * W
KW = 7
NK = 3  # contraction chunks of 128
NM = 3  # output-channel chunks of 128
NTILE = 512
N_CORES = 8
SB = B // N_CORES  # batches per core
G = 2  # pipeline groups per core
BG = SB // G  # batches per group
FREE = BG * HW  # free dim per tile

_CACHE = {}


def _perm_and_segs():
    """Residue-grouped channel permutation and per-chunk DMA segments.

    perm: channels ordered by residue r = c % 7 (stable).  All channels of
    residue r share offset d = (r+3)%7-3.  segs[k] = [(p0, p1, d)] within
    chunk k (partitions [128k+p0, 128k+p1), contiguous in permuted DRAM).
    """
    perm = [c for r in range(KW) for c in range(C) if c % KW == r]
    segs = [[] for _ in range(NK)]
    i = 0
    for r in range(KW):
        cnt = len([c for c in range(C) if c % KW == r])
        d = (r + 3) % KW - KW // 2
        lo = i
        hi = i + cnt
        while lo < hi:
            k = lo // 128
            seg_hi = min(hi, (k + 1) * 128)
            segs[k].append((lo - 128 * k, seg_hi - 128 * k, d))
            lo = seg_hi
        i = hi
    return np.asarray(perm), segs


_PERM, _SEGS = _perm_and_segs()


def _prep_weights(weight, bias):
    import ml_dtypes

    wb = np.sign(weight.astype(np.float32))  # [O, C]
    lhsT = np.ascontiguousarray(wb.T[_PERM, :])  # [C_perm, O]
    wt = np.ascontiguousarray(lhsT.reshape(NK, 128, C).transpose(1, 0, 2)).astype(
        ml_dtypes.bfloat16
    )  # [128, NK, C]
    bias_sb = np.ascontiguousarray(bias.astype(np.float32).reshape(NM, 128).T)

    mask = np.ones((128, NK, W), dtype=np.float32)
    for k in range(NK):
        for (p0, p1, d) in _SEGS[k]:
            if d > 0:
                mask[p0:p1, k, W - d : W] = 0.0
            elif d < 0:
                mask[p0:p1, k, 0:-d] = 0.0
    mask = mask.astype(ml_dtypes.bfloat16)
    return wt, bias_sb, mask


def _legalize_waits(nc, max_waits=1):
    """Walrus for this toolchain accepts at most one sem wait per
    instruction.  Split instructions carrying more into preceding
    same-engine NoOps (engine streams are in-order, so the split is
    semantically identical to the combined wait)."""
    import concourse.mybir as mybir

    fn = nc.m.functions[0]
    ctr = 0
    for blk in fn.blocks:
        out = []
        changed = False
        for inst in blk.instructions:
            si = inst.sync_info
            waits = list(si.on_wait) if si is not None and si.on_wait else []
            if len(waits) > max_waits and str(inst.engine) != "EngineType.Unassigned":
                keep = waits[-max_waits:]
                extra = waits[:-max_waits]
                for j in range(0, len(extra), max_waits):
                    nop = mybir.InstNoOp(name=f"I-waitsplit-{ctr}")
                    ctr += 1
                    nop.engine = inst.engine
                    nop.sync_info = mybir.SyncInfo(
                        on_wait=extra[j : j + max_waits], on_update=[]
                    )
                    out.append(nop)
                si.on_wait = keep
                changed = True
            out.append(inst)
        if changed:
            blk.instructions = out
    return ctr


def _build(raw_bufs=3, psum_bufs=6, ost_bufs=3, g_bufs=2, legalize=True):
    import concourse.bass as bass
    import concourse.mybir as mybir
    import concourse.tile as tile
    from concourse.ap import AP

    nc = bass.Bass()
    x_d = nc.declare_dram_parameter("x", [G, C, FREE], mybir.dt.bfloat16, isOutput=False)
    wt_d = nc.declare_dram_parameter("wt", [128, NK, C], mybir.dt.bfloat16, isOutput=False)
    bias_d = nc.declare_dram_parameter("bias", [128, NM], mybir.dt.float32, isOutput=False)
    mask_d = nc.declare_dram_parameter("mask", [128, NK, W], mybir.dt.bfloat16, isOutput=False)
    out_d = nc.declare_dram_parameter("out", [G, C, FREE], mybir.dt.bfloat16, isOutput=True)

    with tile.TileContext(nc) as tc:
        with (
            tc.tile_pool(name="const", bufs=1) as const_pool,
            tc.tile_pool(name="raw", bufs=raw_bufs) as raw_pool,
            tc.tile_pool(name="g", bufs=g_bufs) as g_pool,
            tc.tile_pool(name="ost", bufs=ost_bufs) as ost_pool,
            tc.tile_pool(name="ps", bufs=psum_bufs, space="PSUM") as ps_pool,
        ):
            wt = const_pool.tile([128, NK, C], mybir.dt.bfloat16)
            bias_sb = const_pool.tile([128, NM], mybir.dt.float32)
            mask_sb = const_pool.tile([128, NK, W], mybir.dt.bfloat16)
            nc.sync.dma_start(wt[:], wt_d[:])
            nc.sync.dma_start(bias_sb[:], bias_d[:])
            nc.sync.dma_start(mask_sb[:], mask_d[:])

            for g in range(G):
                gt = []
                for k in range(NK):
                    raw = raw_pool.tile([128, FREE], mybir.dt.bfloat16, tag="raw")
                    for (p0, p1, d) in _SEGS[k]:
                        src = AP(
                            tensor=x_d,
                            offset=g * C * FREE + (128 * k + p0) * FREE + d,
                            ap=[[FREE, p1 - p0], [1, FREE]],
                        )
                        nc.sync.dma_start(raw[p0:p1, :], src)
                    gk = g_pool.tile([128, FREE], mybir.dt.bfloat16, tag=f"g{k}")
                    half = FREE // 2
                    nc.scalar.sign(gk[:, 0:half], raw[:, 0:half])
                    nc.scalar.sign(gk[:, half:FREE], raw[:, half:FREE])
                    # zero the columns whose shifted source fell outside the
                    # row (the DMA leaked neighbor-row data there) via a
                    # broadcast bf16 mask multiply
                    v = gk.rearrange("p (r w) -> p r w", w=W)
                    mk = mask_sb[:, k : k + 1, :].broadcast_to([128, BG * H, W])
                    nc.vector.tensor_mul(v, v, mk)
                    gt.append(gk)

                for m in range(NM):
                    ost = ost_pool.tile([128, FREE], mybir.dt.bfloat16, tag="ost")
                    for n in range(FREE // NTILE):
                        ps = ps_pool.tile([128, NTILE], mybir.dt.float32, tag="ps")
                        for k in range(NK):
                            nc.tensor.matmul(
                                ps[:],
                                wt[:, k, m * 128 : (m + 1) * 128],
                                gt[k][:, n * NTILE : (n + 1) * NTILE],
                                start=(k == 0),
                                stop=(k == NK - 1),
                            )
                        nc.vector.tensor_scalar_add(
                            ost[:, n * NTILE : (n + 1) * NTILE],
                            ps[:],
                            bias_sb[:, m : m + 1],
                        )
                    # stores go out on the ACT HWDGE ring to split sequencer
                    # issue load between the two rings
                    nc.scalar.dma_start(
                        out_d[g, m * 128 : (m + 1) * 128, :], ost[:]
                    )
    if legalize:
        _legalize_waits(nc)
    return nc


def _ensure_ntff_hook():
    """Register the axon NTFF profiling hook if the image's antenv lacks it."""
    import types

    try:
        from antenv.axon_hooks import get_axon_ntff_profile_hook  # noqa: F401

        return
    except ImportError:
        pass
    hook = None
    try:
        from trn_agent_boot.trn_boot import _ntff_profile_via_ctypes

        hook = _ntff_profile_via_ctypes("/opt/axon/libaxon_pjrt.so")
    except Exception:
        pass
    mod = types.ModuleType("antenv.axon_hooks")
    mod._hook = hook
    mod.get_axon_ntff_profile_hook = lambda: mod._hook
    mod.set_axon_ntff_profile_hook = lambda h: setattr(mod, "_hook", h)
    sys.modules["antenv.axon_hooks"] = mod
    try:
        import antenv

        antenv.axon_hooks = mod
    except Exception:
        pass


def run(x, weight, bias, trace=False):
    """Returns (out [B,C,H,W] f32, exec_time_ns or None)."""
    import ml_dtypes
    import concourse.bass_utils as bu
    from concourse.bass_utils import run_bass_kernel_spmd

    if trace:
        _ensure_ntff_hook()
        # zero-egress container: don't try to copy trace artifacts to a bucket
        bu.upload_artifacts = lambda tmpdir: tmpdir

    if "nc" not in _CACHE:
        _CACHE["nc"] = _build()
    nc = _CACHE["nc"]

    wt, bias_sb, mask = _prep_weights(weight, bias)
    # permute channels into residue-grouped order, cast to bf16 (sign-exact)
    xq = np.asarray(x, dtype=np.float32).reshape(B, C, HW)[:, _PERM, :].astype(
        ml_dtypes.bfloat16
    )
    in_maps = []
    for i in range(N_CORES):
        xs = xq[i * SB : (i + 1) * SB]  # [SB, C, HW]
        # [G, C, BG*HW] channel-major per group so segment loads are contiguous
        xg = np.ascontiguousarray(
            xs.reshape(G, BG, C, HW).transpose(0, 2, 1, 3)
        ).reshape(G, C, FREE)
        in_maps.append({"x": xg, "wt": wt, "bias": bias_sb, "mask": mask})
    res = run_bass_kernel_spmd(
        nc, in_maps, core_ids=list(range(N_CORES)), trace=trace
    )
    outs = []
    for i in range(N_CORES):
        o = np.asarray(res.results[i]["out"]).reshape(G, C, BG, HW)
        outs.append(o.transpose(0, 2, 1, 3).reshape(SB, C, HW))
    out = np.concatenate(outs, axis=0).astype(np.float32)
    return out.reshape(B, C, H, W), res.exec_time_ns


def kernel(x, weight, bias):
    out, _ = run(x, weight, bias, trace=False)
    return out


# revision 9
# speedup vs baseline: 2.7177x; 1.2319x over previous
"""CycleMLP 1w1a (binary cycle-shift conv + 1x1 GEMM) for 8 Trainium2 cores.

  out[b,o,h,w] = sum_c sign(weight)[o,c] * sign(x)[b,c,h,w+off(c)] + bias[o]
  off(c) = (c+3) % 7 - 3, zero-padded outside [0, W)

Sharding: data-parallel over batch B=64 -> 8 batches/core; weight/bias
replicated.

Key layout tricks (all host-side, mathematically identity):
  - channels permuted so shift-groups (residue c % 7) are contiguous and
    ordered by DESCENDING shift d; the weight's contraction dim is permuted
    identically.  With one pad element inserted between groups in the DRAM
    buffer, consecutive shifted per-channel windows tile the buffer exactly
    contiguously, so each 128-channel chunk loads as ONE dense full-partition
    2D DMA (the only transfer shape that fans evenly across all 16 SDMA
    engines; sub-128-partition or lattice transfers pile onto engine 0).
    The shift itself is still performed by the device DMA via the base
    offset; all group/row-boundary leaks land in masked columns.
  - per-core layout [G, C*BG*HW] (channel-major per half-batch group):
    6 input loads + 6 stores of ~1 MB each per core, total.
  - bf16 x (sign-preserving) and bf16 out (integer sums <= 384 round with
    ulp 0.125 -> rel err ~5e-4) halve HBM traffic.

Per-core kernel, phase-ordered so no engine stream blocks another:
  all loads (Sync HWDGE) | sign per chunk (ScalarE) | boundary mask-mul
  (GpSimd) | GEMM 3m x 8n x 3k with PSUM K-accum (TensorE) | bias-add
  eviction to bf16 (DVE) | stores (Sync HWDGE, queued after all loads).
"""

import sys

for p in ("/opt/trn_rl_repo", "/root/.axon_site/_ro/trn_rl_repo"):
    if p not in sys.path:
        sys.path.append(p)

import numpy as np

B = 64
C = 384
H = W = 32
HW = H * W
KW = 7
NK = 3  # contraction chunks of 128
NM = 3  # output-channel chunks of 128
NTILE = 512
N_CORES = 8
SB = B // N_CORES  # batches per core
G = 2  # pipeline groups per core
BG = SB // G  # batches per group
FREE = BG * HW  # free dim per tile
LEAD = 3  # read-base offset = d of the first (largest-d) group
PADLEN = C * FREE + 2 * LEAD  # + one pad elem between the 7 groups

_CACHE = {}

# residues ordered by descending shift d = (r+3)%7-3
_RES_ORDER = sorted(range(KW), key=lambda r: -((r + 3) % KW - KW // 2))


def _perm_and_segs():
    """Channel permutation (residue groups, descending d) and per-chunk
    segments segs[k] = [(p0, p1, d)] (partitions [128k+p0, 128k+p1))."""
    perm = []
    segs = [[] for _ in range(NK)]
    i = 0
    for r in _RES_ORDER:
        chans = [c for c in range(C) if c % KW == r]
        perm.extend(chans)
        d = (r + 3) % KW - KW // 2
        lo, hi = i, i + len(chans)
        while lo < hi:
            k = lo // 128
            seg_hi = min(hi, (k + 1) * 128)
            segs[k].append((lo - 128 * k, seg_hi - 128 * k, d))
            lo = seg_hi
        i = hi
    return np.asarray(perm), segs


_PERM, _SEGS = _perm_and_segs()
# group boundaries in permuted index space (for pad insertion)
_GROUP_SIZES = [len([c for c in range(C) if c % KW == r]) for r in _RES_ORDER]


def _prep_weights(weight, bias):
    import ml_dtypes

    wb = np.sign(weight.astype(np.float32))  # [O, C]
    lhsT = np.ascontiguousarray(wb.T[_PERM, :])  # [C_perm, O]
    wt = np.ascontiguousarray(lhsT.reshape(NK, 128, C).transpose(1, 0, 2)).astype(
        ml_dtypes.bfloat16
    )  # [128, NK, C]
    bias_sb = np.ascontiguousarray(bias.astype(np.float32).reshape(NM, 128).T)

    mask = np.ones((128, NK, W), dtype=np.float32)
    for k in range(NK):
        for (p0, p1, d) in _SEGS[k]:
            if d > 0:
                mask[p0:p1, k, W - d : W] = 0.0
            elif d < 0:
                mask[p0:p1, k, 0:-d] = 0.0
    mask = mask.astype(ml_dtypes.bfloat16)
    return wt, bias_sb, mask


def _legalize_waits(nc, max_waits=1):
    """Walrus for this toolchain accepts at most one sem wait per
    instruction.  Split instructions carrying more into preceding
    same-engine NoOps (engine streams are in-order, so the split is
    semantically identical to the combined wait)."""
    import concourse.mybir as mybir

    fn = nc.m.functions[0]
    ctr = 0
    for blk in fn.blocks:
        out = []
        changed = False
        for inst in blk.instructions:
            si = inst.sync_info
            waits = list(si.on_wait) if si is not None and si.on_wait else []
            if len(waits) > max_waits and str(inst.engine) != "EngineType.Unassigned":
                keep = waits[-max_waits:]
                extra = waits[:-max_waits]
                for j in range(0, len(extra), max_waits):
                    nop = mybir.InstNoOp(name=f"I-waitsplit-{ctr}")
                    ctr += 1
                    nop.engine = inst.engine
                    nop.sync_info = mybir.SyncInfo(
                        on_wait=extra[j : j + max_waits], on_update=[]
                    )
                    out.append(nop)
                si.on_wait = keep
                changed = True
            out.append(inst)
        if changed:
            blk.instructions = out
    return ctr


def _build(psum_bufs=6, ost_bufs=4, legalize=True):
    import concourse.bass as bass
    import concourse.mybir as mybir
    import concourse.tile as tile
    from concourse.ap import AP

    nc = bass.Bass()
    x_d = nc.declare_dram_parameter("x", [G, PADLEN], mybir.dt.bfloat16, isOutput=False)
    wt_d = nc.declare_dram_parameter("wt", [128, NK, C], mybir.dt.bfloat16, isOutput=False)
    bias_d = nc.declare_dram_parameter("bias", [128, NM], mybir.dt.float32, isOutput=False)
    mask_d = nc.declare_dram_parameter("mask", [128, NK, W], mybir.dt.bfloat16, isOutput=False)
    out_d = nc.declare_dram_parameter("out", [G, C, FREE], mybir.dt.bfloat16, isOutput=True)

    with tile.TileContext(nc) as tc:
        with (
            tc.tile_pool(name="const", bufs=1) as const_pool,
            tc.tile_pool(name="raw", bufs=G * NK) as raw_pool,
            tc.tile_pool(name="g", bufs=G) as g_pool,
            tc.tile_pool(name="ost", bufs=ost_bufs) as ost_pool,
            tc.tile_pool(name="ps", bufs=psum_bufs, space="PSUM") as ps_pool,
        ):
            wt = const_pool.tile([128, NK, C], mybir.dt.bfloat16)
            bias_sb = const_pool.tile([128, NM], mybir.dt.float32)
            mask_sb = const_pool.tile([128, NK, W], mybir.dt.bfloat16)
            nc.sync.dma_start(wt[:], wt_d[:])
            nc.sync.dma_start(bias_sb[:], bias_d[:])
            nc.sync.dma_start(mask_sb[:], mask_d[:])

            # phase L: all input loads up front (dense full-partition 2D)
            raws = {}
            for g in range(G):
                for k in range(NK):
                    raw = raw_pool.tile([128, FREE], mybir.dt.bfloat16, tag="raw")
                    src = AP(
                        tensor=x_d,
                        offset=g * PADLEN + LEAD + k * 128 * FREE,
                        ap=[[FREE, 128], [1, FREE]],
                    )
                    nc.sync.dma_start(raw[:], src)
                    raws[(g, k)] = raw

            # phase C: sign (ScalarE, split for pipelining) + boundary mask
            # multiply (GpSimd, broadcast over rows)
            gts = {}
            half = FREE // 2
            for g in range(G):
                for k in range(NK):
                    raw = raws[(g, k)]
                    gk = g_pool.tile([128, FREE], mybir.dt.bfloat16, tag=f"g{k}")
                    nc.scalar.sign(gk[:, 0:half], raw[:, 0:half])
                    nc.scalar.sign(gk[:, half:FREE], raw[:, half:FREE])
                    v = gk.rearrange("p (r w) -> p r w", w=W)
                    mk = mask_sb[:, k : k + 1, :].broadcast_to([128, BG * H, W])
                    nc.gpsimd.tensor_mul(v, v, mk)
                    gts[(g, k)] = gk

            # phase M: GEMM + bias eviction + stores (stores on the Sync ring,
            # emitted after every load so they never block a load issue)
            for g in range(G):
                for m in range(NM):
                    ost = ost_pool.tile([128, FREE], mybir.dt.bfloat16, tag="ost")
                    for n in range(FREE // NTILE):
                        ps = ps_pool.tile([128, NTILE], mybir.dt.float32, tag="ps")
                        for k in range(NK):
                            nc.tensor.matmul(
                                ps[:],
                                wt[:, k, m * 128 : (m + 1) * 128],
                                gts[(g, k)][:, n * NTILE : (n + 1) * NTILE],
                                start=(k == 0),
                                stop=(k == NK - 1),
                            )
                        nc.vector.tensor_scalar_add(
                            ost[:, n * NTILE : (n + 1) * NTILE],
                            ps[:],
                            bias_sb[:, m : m + 1],
                        )
                    nc.sync.dma_start(
                        out_d[g, m * 128 : (m + 1) * 128, :], ost[:]
                    )
    if legalize:
        _legalize_waits(nc)
    return nc


def _ensure_ntff_hook():
    """Register the axon NTFF profiling hook if the image's antenv lacks it."""
    import types

    try:
        from antenv.axon_hooks import get_axon_ntff_profile_hook  # noqa: F401

        return
    except ImportError:
        pass
    hook = None
    try:
        from trn_agent_boot.trn_boot import _ntff_profile_via_ctypes

        hook = _ntff_profile_via_ctypes("/opt/axon/libaxon_pjrt.so")
    except Exception:
        pass
    mod = types.ModuleType("antenv.axon_hooks")
    mod._hook = hook
    mod.get_axon_ntff_profile_hook = lambda: mod._hook
    mod.set_axon_ntff_profile_hook = lambda h: setattr(mod, "_hook", h)
    sys.modules["antenv.axon_hooks"] = mod
    try:
        import antenv

        antenv.axon_hooks = mod
    except Exception:
        pass


def _pack_x(x):
    """[B, C, H, W] f32 -> per-core [G, PADLEN] bf16 buffers with the
    permuted channel-major layout and 1-elem pads between shift groups."""
    import ml_dtypes

    xq = np.asarray(x, dtype=np.float32).reshape(B, C, HW)[:, _PERM, :].astype(
        ml_dtypes.bfloat16
    )
    shards = []
    for i in range(N_CORES):
        buf = np.zeros((G, PADLEN), dtype=ml_dtypes.bfloat16)
        for g in range(G):
            src = xq[i * SB + g * BG : i * SB + (g + 1) * BG]  # [BG, C, HW]
            xt = np.ascontiguousarray(src.transpose(1, 0, 2)).reshape(C, FREE)
            pos = 0
            a = 0
            for n in _GROUP_SIZES:
                buf[g, pos : pos + n * FREE] = xt[a : a + n].reshape(-1)
                pos += n * FREE + 1
                a += n
        shards.append(buf)
    return shards


def run(x, weight, bias, trace=False):
    """Returns (out [B,C,H,W] f32, exec_time_ns or None)."""
    import concourse.bass_utils as bu
    from concourse.bass_utils import run_bass_kernel_spmd

    if trace:
        _ensure_ntff_hook()
        # zero-egress container: don't try to copy trace artifacts to a bucket
        bu.upload_artifacts = lambda tmpdir: tmpdir

    if "nc" not in _CACHE:
        _CACHE["nc"] = _build()
    nc = _CACHE["nc"]

    wt, bias_sb, mask = _prep_weights(weight, bias)
    shards = _pack_x(x)
    in_maps = [
        {"x": shards[i], "wt": wt, "bias": bias_sb, "mask": mask}
        for i in range(N_CORES)
    ]
    res = run_bass_kernel_spmd(
        nc, in_maps, core_ids=list(range(N_CORES)), trace=trace
    )
    outs = []
    for i in range(N_CORES):
        o = np.asarray(res.results[i]["out"]).reshape(G, C, BG, HW)
        outs.append(o.transpose(0, 2, 1, 3).reshape(SB, C, HW))
    out = np.concatenate(outs, axis=0).astype(np.float32)
    return out.reshape(B, C, H, W), res.exec_time_ns


def kernel(x, weight, bias):
    out, _ = run(x, weight, bias, trace=False)
    return out


# revision 14
# speedup vs baseline: 4.3193x; 1.5893x over previous
"""CycleMLP 1w1a (binary cycle-shift conv + 1x1 GEMM) for 8 Trainium2 cores.

  out[b,o,h,w] = sum_c sign(weight)[o,c] * sign(x)[b,c,h,w+off(c)] + bias[o]
  off(c) = (c+3) % 7 - 3, zero-padded outside [0, W)

Sharding: data-parallel over batch B=64 -> 8 batches/core; weight/bias
replicated.

Key layout tricks (all host-side, mathematically identity):
  - channels permuted so shift-groups (residue c % 7) are contiguous and
    ordered by DESCENDING shift d; the weight's contraction dim is permuted
    identically.  With one pad element inserted between groups in the DRAM
    buffer, consecutive shifted per-channel windows tile the buffer exactly
    contiguously, so each 128-channel chunk loads as ONE dense full-partition
    2D DMA (the only transfer shape that fans evenly across all 16 SDMA
    engines; sub-128-partition or lattice transfers pile onto engine 0).
    The shift itself is still performed by the device DMA via the base
    offset; all group/row-boundary leaks land in masked columns.
  - per-core layout [G, C*BG*HW] (channel-major per half-batch group):
    6 input loads + 6 stores of ~1 MB each per core, total.
  - bf16 x (sign-preserving) and bf16 out (integer sums <= 384 round with
    ulp 0.125 -> rel err ~5e-4) halve HBM traffic.

Per-core kernel, phase-ordered so no engine stream blocks another:
  all loads (Sync HWDGE) | sign per chunk (ScalarE) | boundary mask-mul
  (GpSimd) | GEMM 3m x 8n x 3k with PSUM K-accum (TensorE) | bias-add
  eviction to bf16 (DVE) | stores (Sync HWDGE, queued after all loads).
"""

import sys

for p in ("/opt/trn_rl_repo", "/root/.axon_site/_ro/trn_rl_repo"):
    if p not in sys.path:
        sys.path.append(p)

import numpy as np

B = 64
C = 384
H = W = 32
HW = H * W
KW = 7
NK = 3  # contraction chunks of 128
NM = 3  # output-channel chunks of 128
NTILE = 512
N_CORES = 8
SB = B // N_CORES  # batches per core
G = 4  # pipeline groups per core
BG = SB // G  # batches per group
FREE = BG * HW  # free dim per tile
LEAD = 3  # read-base offset = d of the first (largest-d) group
PADLEN = C * FREE + 2 * LEAD  # + one pad elem between the 7 groups

_CACHE = {}

# residues ordered by descending shift d = (r+3)%7-3
_RES_ORDER = sorted(range(KW), key=lambda r: -((r + 3) % KW - KW // 2))


def _perm_and_segs():
    """Channel permutation (residue groups, descending d) and per-chunk
    segments segs[k] = [(p0, p1, d)] (partitions [128k+p0, 128k+p1))."""
    perm = []
    segs = [[] for _ in range(NK)]
    i = 0
    for r in _RES_ORDER:
        chans = [c for c in range(C) if c % KW == r]
        perm.extend(chans)
        d = (r + 3) % KW - KW // 2
        lo, hi = i, i + len(chans)
        while lo < hi:
            k = lo // 128
            seg_hi = min(hi, (k + 1) * 128)
            segs[k].append((lo - 128 * k, seg_hi - 128 * k, d))
            lo = seg_hi
        i = hi
    return np.asarray(perm), segs


_PERM, _SEGS = _perm_and_segs()
# group boundaries in permuted index space (for pad insertion)
_GROUP_SIZES = [len([c for c in range(C) if c % KW == r]) for r in _RES_ORDER]


def _prep_weights(weight, bias):
    import ml_dtypes

    # weights scaled to +-2: the device computes g = (x >= 0) - 0.5 in {+-0.5},
    # so products are +-1 and PSUM sums match the +-1 x +-1 reference exactly
    wb = np.sign(weight.astype(np.float32)) * 2.0  # [O, C]
    lhsT = np.ascontiguousarray(wb.T[_PERM, :])  # [C_perm, O]
    wt = np.ascontiguousarray(lhsT.reshape(NK, 128, C).transpose(1, 0, 2)).astype(
        ml_dtypes.bfloat16
    )  # [128, NK, C]
    bias_sb = np.ascontiguousarray(bias.astype(np.float32).reshape(NM, 128).T)

    mask = np.ones((128, NK, W), dtype=np.float32)
    for k in range(NK):
        for (p0, p1, d) in _SEGS[k]:
            if d > 0:
                mask[p0:p1, k, W - d : W] = 0.0
            elif d < 0:
                mask[p0:p1, k, 0:-d] = 0.0
    mask = mask.astype(ml_dtypes.bfloat16)
    return wt, bias_sb, mask


def _legalize_waits(nc, max_waits=1):
    """Walrus for this toolchain accepts at most one sem wait per
    instruction.  Split instructions carrying more into preceding
    same-engine NoOps (engine streams are in-order, so the split is
    semantically identical to the combined wait)."""
    import concourse.mybir as mybir

    fn = nc.m.functions[0]
    ctr = 0
    for blk in fn.blocks:
        out = []
        changed = False
        for inst in blk.instructions:
            si = inst.sync_info
            waits = list(si.on_wait) if si is not None and si.on_wait else []
            if len(waits) > max_waits and str(inst.engine) != "EngineType.Unassigned":
                keep = waits[-max_waits:]
                extra = waits[:-max_waits]
                for j in range(0, len(extra), max_waits):
                    nop = mybir.InstNoOp(name=f"I-waitsplit-{ctr}")
                    ctr += 1
                    nop.engine = inst.engine
                    nop.sync_info = mybir.SyncInfo(
                        on_wait=extra[j : j + max_waits], on_update=[]
                    )
                    out.append(nop)
                si.on_wait = keep
                changed = True
            out.append(inst)
        if changed:
            blk.instructions = out
    return ctr


def _build(psum_bufs=8, ost_bufs=4, legalize=True):
    import concourse.bass as bass
    import concourse.mybir as mybir
    import concourse.tile as tile
    from concourse.ap import AP

    nc = bass.Bass()
    x_d = nc.declare_dram_parameter("x", [G, PADLEN], mybir.dt.bfloat16, isOutput=False)
    wt_d = nc.declare_dram_parameter("wt", [128, NK, C], mybir.dt.bfloat16, isOutput=False)
    bias_d = nc.declare_dram_parameter("bias", [128, NM], mybir.dt.float32, isOutput=False)
    mask_d = nc.declare_dram_parameter("mask", [128, NK, W], mybir.dt.bfloat16, isOutput=False)
    out_d = nc.declare_dram_parameter("out", [G, C, FREE], mybir.dt.bfloat16, isOutput=True)

    with tile.TileContext(nc) as tc:
        with (
            tc.tile_pool(name="const", bufs=1) as const_pool,
            tc.tile_pool(name="raw", bufs=G * NK) as raw_pool,
            tc.tile_pool(name="g", bufs=G) as g_pool,
            tc.tile_pool(name="ost", bufs=ost_bufs) as ost_pool,
            tc.tile_pool(name="ps", bufs=psum_bufs, space="PSUM") as ps_pool,
        ):
            wt = const_pool.tile([128, NK, C], mybir.dt.bfloat16)
            bias_sb = const_pool.tile([128, NM], mybir.dt.float32)
            mask_sb = const_pool.tile([128, NK, W], mybir.dt.bfloat16)
            nc.sync.dma_start(wt[:], wt_d[:])
            nc.sync.dma_start(bias_sb[:], bias_d[:])
            nc.sync.dma_start(mask_sb[:], mask_d[:])

            # phase L: all input loads up front (dense full-partition 2D)
            raws = {}
            for g in range(G):
                for k in range(NK):
                    raw = raw_pool.tile([128, FREE], mybir.dt.bfloat16, tag="raw")
                    src = AP(
                        tensor=x_d,
                        offset=g * PADLEN + LEAD + k * 128 * FREE,
                        ap=[[FREE, 128], [1, FREE]],
                    )
                    nc.sync.dma_start(raw[:], src)
                    raws[(g, k)] = raw

            # phase C: binarize g = (x >= 0) - 0.5 in {+-0.5} (one fused
            # two-op tensor_scalar on DVE) + boundary mask multiply (DVE)
            gts = {}
            for g in range(G):
                for k in range(NK):
                    raw = raws[(g, k)]
                    gk = g_pool.tile([128, FREE], mybir.dt.bfloat16, tag=f"g{k}")
                    nc.vector.tensor_scalar(
                        gk[:],
                        raw[:],
                        0.0,
                        0.5,
                        mybir.AluOpType.is_ge,
                        mybir.AluOpType.subtract,
                    )
                    v = gk.rearrange("p (r w) -> p r w", w=W)
                    mk = mask_sb[:, k : k + 1, :].broadcast_to([128, BG * H, W])
                    nc.vector.tensor_mul(v, v, mk)
                    gts[(g, k)] = gk

            # phase M: GEMM (k-outer so consecutive matmuls share weights) +
            # bias eviction on ScalarE + stores (Sync ring, emitted after
            # every load so they never block a load issue)
            NT = FREE // NTILE
            for g in range(G):
                for m in range(NM):
                    pss = []
                    for _n in range(NT):
                        ps = ps_pool.tile(
                            [128, NTILE], mybir.dt.float32, tag="ps", name=f"ps{_n}"
                        )
                        pss.append(ps)
                    for k in range(NK):
                        for n in range(NT):
                            nc.tensor.matmul(
                                pss[n][:],
                                wt[:, k, m * 128 : (m + 1) * 128],
                                gts[(g, k)][:, n * NTILE : (n + 1) * NTILE],
                                start=(k == 0),
                                stop=(k == NK - 1),
                            )
                    ost = ost_pool.tile([128, FREE], mybir.dt.bfloat16, tag="ost")
                    for n in range(NT):
                        nc.scalar.activation(
                            ost[:, n * NTILE : (n + 1) * NTILE],
                            pss[n][:],
                            mybir.ActivationFunctionType.Identity,
                            bias=bias_sb[:, m : m + 1],
                        )
                    nc.sync.dma_start(
                        out_d[g, m * 128 : (m + 1) * 128, :], ost[:]
                    )
    if legalize:
        _legalize_waits(nc)
    return nc


def _ensure_ntff_hook():
    """Register the axon NTFF profiling hook if the image's antenv lacks it."""
    import types

    try:
        from antenv.axon_hooks import get_axon_ntff_profile_hook  # noqa: F401

        return
    except ImportError:
        pass
    hook = None
    try:
        from trn_agent_boot.trn_boot import _ntff_profile_via_ctypes

        hook = _ntff_profile_via_ctypes("/opt/axon/libaxon_pjrt.so")
    except Exception:
        pass
    mod = types.ModuleType("antenv.axon_hooks")
    mod._hook = hook
    mod.get_axon_ntff_profile_hook = lambda: mod._hook
    mod.set_axon_ntff_profile_hook = lambda h: setattr(mod, "_hook", h)
    sys.modules["antenv.axon_hooks"] = mod
    try:
        import antenv

        antenv.axon_hooks = mod
    except Exception:
        pass


def _pack_x(x):
    """[B, C, H, W] f32 -> per-core [G, PADLEN] bf16 buffers with the
    permuted channel-major layout and 1-elem pads between shift groups."""
    import ml_dtypes

    xq = np.asarray(x, dtype=np.float32).reshape(B, C, HW)[:, _PERM, :].astype(
        ml_dtypes.bfloat16
    )
    shards = []
    for i in range(N_CORES):
        buf = np.zeros((G, PADLEN), dtype=ml_dtypes.bfloat16)
        for g in range(G):
            src = xq[i * SB + g * BG : i * SB + (g + 1) * BG]  # [BG, C, HW]
            xt = np.ascontiguousarray(src.transpose(1, 0, 2)).reshape(C, FREE)
            pos = 0
            a = 0
            for n in _GROUP_SIZES:
                buf[g, pos : pos + n * FREE] = xt[a : a + n].reshape(-1)
                pos += n * FREE + 1
                a += n
        shards.append(buf)
    return shards


def run(x, weight, bias, trace=False):
    """Returns (out [B,C,H,W] f32, exec_time_ns or None)."""
    import concourse.bass_utils as bu
    from concourse.bass_utils import run_bass_kernel_spmd

    if trace:
        _ensure_ntff_hook()
        # zero-egress container: don't try to copy trace artifacts to a bucket
        bu.upload_artifacts = lambda tmpdir: tmpdir

    if "nc" not in _CACHE:
        _CACHE["nc"] = _build()
    nc = _CACHE["nc"]

    wt, bias_sb, mask = _prep_weights(weight, bias)
    shards = _pack_x(x)
    in_maps = [
        {"x": shards[i], "wt": wt, "bias": bias_sb, "mask": mask}
        for i in range(N_CORES)
    ]
    res = run_bass_kernel_spmd(
        nc, in_maps, core_ids=list(range(N_CORES)), trace=trace
    )
    outs = []
    for i in range(N_CORES):
        o = np.asarray(res.results[i]["out"]).reshape(G, C, BG, HW)
        outs.append(o.transpose(0, 2, 1, 3).reshape(SB, C, HW))
    out = np.concatenate(outs, axis=0).astype(np.float32)
    return out.reshape(B, C, H, W), res.exec_time_ns


def kernel(x, weight, bias):
    out, _ = run(x, weight, bias, trace=False)
    return out


# revision 15
# speedup vs baseline: 4.3493x; 1.0069x over previous
"""CycleMLP 1w1a (binary cycle-shift conv + 1x1 GEMM) for 8 Trainium2 cores.

  out[b,o,h,w] = sum_c sign(weight)[o,c] * sign(x)[b,c,h,w+off(c)] + bias[o]
  off(c) = (c+3) % 7 - 3, zero-padded outside [0, W)

Sharding: data-parallel over batch B=64 -> 8 batches/core; weight/bias
replicated.

Key layout tricks (all host-side, mathematically identity):
  - channels permuted so shift-groups (residue c % 7) are contiguous and
    ordered by DESCENDING shift d; the weight's contraction dim is permuted
    identically.  With one pad element inserted between groups in the DRAM
    buffer, consecutive shifted per-channel windows tile the buffer exactly
    contiguously, so each 128-channel chunk loads as ONE dense full-partition
    2D DMA (the only transfer shape that fans evenly across all 16 SDMA
    engines; sub-128-partition or lattice transfers pile onto engine 0).
    The shift itself is still performed by the device DMA via the base
    offset; all group/row-boundary leaks land in masked columns.
  - per-core layout [G, C*BG*HW] (channel-major per half-batch group):
    6 input loads + 6 stores of ~1 MB each per core, total.
  - fp8e5m2 x (sign-preserving except |x| < 2^-17, P ~ 3e-6) and bf16 out
    (integer sums <= 384 round with ulp 0.125 -> rel err ~5e-4) cut HBM
    traffic to 0.25x + 0.5x of f32.

Per-core kernel, phase-ordered so no engine stream blocks another:
  all loads (Sync HWDGE) | sign per chunk (ScalarE) | boundary mask-mul
  (GpSimd) | GEMM 3m x 8n x 3k with PSUM K-accum (TensorE) | bias-add
  eviction to bf16 (DVE) | stores (Sync HWDGE, queued after all loads).
"""

import sys

for p in ("/opt/trn_rl_repo", "/root/.axon_site/_ro/trn_rl_repo"):
    if p not in sys.path:
        sys.path.append(p)

import numpy as np

B = 64
C = 384
H = W = 32
HW = H * W
KW = 7
NK = 3  # contraction chunks of 128
NM = 3  # output-channel chunks of 128
NTILE = 512
N_CORES = 8
SB = B // N_CORES  # batches per core
G = 4  # pipeline groups per core
BG = SB // G  # batches per group
FREE = BG * HW  # free dim per tile
LEAD = 3  # read-base offset = d of the first (largest-d) group
PADLEN = C * FREE + 2 * LEAD  # + one pad elem between the 7 groups

_CACHE = {}

# residues ordered by descending shift d = (r+3)%7-3
_RES_ORDER = sorted(range(KW), key=lambda r: -((r + 3) % KW - KW // 2))


def _perm_and_segs():
    """Channel permutation (residue groups, descending d) and per-chunk
    segments segs[k] = [(p0, p1, d)] (partitions [128k+p0, 128k+p1))."""
    perm = []
    segs = [[] for _ in range(NK)]
    i = 0
    for r in _RES_ORDER:
        chans = [c for c in range(C) if c % KW == r]
        perm.extend(chans)
        d = (r + 3) % KW - KW // 2
        lo, hi = i, i + len(chans)
        while lo < hi:
            k = lo // 128
            seg_hi = min(hi, (k + 1) * 128)
            segs[k].append((lo - 128 * k, seg_hi - 128 * k, d))
            lo = seg_hi
        i = hi
    return np.asarray(perm), segs


_PERM, _SEGS = _perm_and_segs()
# group boundaries in permuted index space (for pad insertion)
_GROUP_SIZES = [len([c for c in range(C) if c % KW == r]) for r in _RES_ORDER]


def _prep_weights(weight, bias):
    import ml_dtypes

    # weights scaled to +-2: the device computes g = (x >= 0) - 0.5 in {+-0.5},
    # so products are +-1 and PSUM sums match the +-1 x +-1 reference exactly
    wb = np.sign(weight.astype(np.float32)) * 2.0  # [O, C]
    lhsT = np.ascontiguousarray(wb.T[_PERM, :])  # [C_perm, O]
    wt = np.ascontiguousarray(lhsT.reshape(NK, 128, C).transpose(1, 0, 2)).astype(
        ml_dtypes.bfloat16
    )  # [128, NK, C]
    bias_sb = np.ascontiguousarray(bias.astype(np.float32).reshape(NM, 128).T)

    mask = np.ones((128, NK, W), dtype=np.float32)
    for k in range(NK):
        for (p0, p1, d) in _SEGS[k]:
            if d > 0:
                mask[p0:p1, k, W - d : W] = 0.0
            elif d < 0:
                mask[p0:p1, k, 0:-d] = 0.0
    mask = mask.astype(ml_dtypes.bfloat16)
    return wt, bias_sb, mask


def _legalize_waits(nc, max_waits=1):
    """Walrus for this toolchain accepts at most one sem wait per
    instruction.  Split instructions carrying more into preceding
    same-engine NoOps (engine streams are in-order, so the split is
    semantically identical to the combined wait)."""
    import concourse.mybir as mybir

    fn = nc.m.functions[0]
    ctr = 0
    for blk in fn.blocks:
        out = []
        changed = False
        for inst in blk.instructions:
            si = inst.sync_info
            waits = list(si.on_wait) if si is not None and si.on_wait else []
            if len(waits) > max_waits and str(inst.engine) != "EngineType.Unassigned":
                keep = waits[-max_waits:]
                extra = waits[:-max_waits]
                for j in range(0, len(extra), max_waits):
                    nop = mybir.InstNoOp(name=f"I-waitsplit-{ctr}")
                    ctr += 1
                    nop.engine = inst.engine
                    nop.sync_info = mybir.SyncInfo(
                        on_wait=extra[j : j + max_waits], on_update=[]
                    )
                    out.append(nop)
                si.on_wait = keep
                changed = True
            out.append(inst)
        if changed:
            blk.instructions = out
    return ctr


def _build(psum_bufs=8, ost_bufs=4, legalize=True):
    import concourse.bass as bass
    import concourse.mybir as mybir
    import concourse.tile as tile
    from concourse.ap import AP

    nc = bass.Bass()
    x_d = nc.declare_dram_parameter("x", [G, PADLEN], mybir.dt.float8e5, isOutput=False)
    wt_d = nc.declare_dram_parameter("wt", [128, NK, C], mybir.dt.bfloat16, isOutput=False)
    bias_d = nc.declare_dram_parameter("bias", [128, NM], mybir.dt.float32, isOutput=False)
    mask_d = nc.declare_dram_parameter("mask", [128, NK, W], mybir.dt.bfloat16, isOutput=False)
    out_d = nc.declare_dram_parameter("out", [G, C, FREE], mybir.dt.bfloat16, isOutput=True)

    with tile.TileContext(nc) as tc:
        with (
            tc.tile_pool(name="const", bufs=1) as const_pool,
            tc.tile_pool(name="raw", bufs=G * NK) as raw_pool,
            tc.tile_pool(name="g", bufs=G) as g_pool,
            tc.tile_pool(name="ost", bufs=ost_bufs) as ost_pool,
            tc.tile_pool(name="ps", bufs=psum_bufs, space="PSUM") as ps_pool,
        ):
            wt = const_pool.tile([128, NK, C], mybir.dt.bfloat16)
            bias_sb = const_pool.tile([128, NM], mybir.dt.float32)
            mask_sb = const_pool.tile([128, NK, W], mybir.dt.bfloat16)
            nc.sync.dma_start(wt[:], wt_d[:])
            nc.sync.dma_start(bias_sb[:], bias_d[:])
            nc.sync.dma_start(mask_sb[:], mask_d[:])

            # phase L: all input loads up front (dense full-partition 2D)
            raws = {}
            for g in range(G):
                for k in range(NK):
                    raw = raw_pool.tile([128, FREE], mybir.dt.float8e5, tag="raw")
                    src = AP(
                        tensor=x_d,
                        offset=g * PADLEN + LEAD + k * 128 * FREE,
                        ap=[[FREE, 128], [1, FREE]],
                    )
                    nc.sync.dma_start(raw[:], src)
                    raws[(g, k)] = raw

            # phase C: binarize g = (x >= 0) - 0.5 in {+-0.5} (one fused
            # two-op tensor_scalar on DVE) + boundary mask multiply (DVE)
            gts = {}
            for g in range(G):
                for k in range(NK):
                    raw = raws[(g, k)]
                    gk = g_pool.tile([128, FREE], mybir.dt.bfloat16, tag=f"g{k}")
                    nc.vector.tensor_scalar(
                        gk[:],
                        raw[:],
                        0.0,
                        0.5,
                        mybir.AluOpType.is_ge,
                        mybir.AluOpType.subtract,
                    )
                    v = gk.rearrange("p (r w) -> p r w", w=W)
                    mk = mask_sb[:, k : k + 1, :].broadcast_to([128, BG * H, W])
                    nc.vector.tensor_mul(v, v, mk)
                    gts[(g, k)] = gk

            # phase M: GEMM (k-outer so consecutive matmuls share weights) +
            # bias eviction on ScalarE + stores (Sync ring, emitted after
            # every load so they never block a load issue)
            NT = FREE // NTILE
            for g in range(G):
                for m in range(NM):
                    pss = []
                    for _n in range(NT):
                        ps = ps_pool.tile(
                            [128, NTILE], mybir.dt.float32, tag="ps", name=f"ps{_n}"
                        )
                        pss.append(ps)
                    for k in range(NK):
                        for n in range(NT):
                            nc.tensor.matmul(
                                pss[n][:],
                                wt[:, k, m * 128 : (m + 1) * 128],
                                gts[(g, k)][:, n * NTILE : (n + 1) * NTILE],
                                start=(k == 0),
                                stop=(k == NK - 1),
                            )
                    ost = ost_pool.tile([128, FREE], mybir.dt.bfloat16, tag="ost")
                    for n in range(NT):
                        nc.scalar.activation(
                            ost[:, n * NTILE : (n + 1) * NTILE],
                            pss[n][:],
                            mybir.ActivationFunctionType.Identity,
                            bias=bias_sb[:, m : m + 1],
                        )
                    nc.sync.dma_start(
                        out_d[g, m * 128 : (m + 1) * 128, :], ost[:]
                    )
    if legalize:
        _legalize_waits(nc)
    return nc


def _ensure_ntff_hook():
    """Register the axon NTFF profiling hook if the image's antenv lacks it."""
    import types

    try:
        from antenv.axon_hooks import get_axon_ntff_profile_hook  # noqa: F401

        return
    except ImportError:
        pass
    hook = None
    try:
        from trn_agent_boot.trn_boot import _ntff_profile_via_ctypes

        hook = _ntff_profile_via_ctypes("/opt/axon/libaxon_pjrt.so")
    except Exception:
        pass
    mod = types.ModuleType("antenv.axon_hooks")
    mod._hook = hook
    mod.get_axon_ntff_profile_hook = lambda: mod._hook
    mod.set_axon_ntff_profile_hook = lambda h: setattr(mod, "_hook", h)
    sys.modules["antenv.axon_hooks"] = mod
    try:
        import antenv

        antenv.axon_hooks = mod
    except Exception:
        pass


def _pack_x(x):
    """[B, C, H, W] f32 -> per-core [G, PADLEN] bf16 buffers with the
    permuted channel-major layout and 1-elem pads between shift groups."""
    import ml_dtypes

    xq = np.asarray(x, dtype=np.float32).reshape(B, C, HW)[:, _PERM, :].astype(
        ml_dtypes.float8_e5m2
    )
    shards = []
    for i in range(N_CORES):
        buf = np.zeros((G, PADLEN), dtype=ml_dtypes.float8_e5m2)
        for g in range(G):
            src = xq[i * SB + g * BG : i * SB + (g + 1) * BG]  # [BG, C, HW]
            xt = np.ascontiguousarray(src.transpose(1, 0, 2)).reshape(C, FREE)
            pos = 0
            a = 0
            for n in _GROUP_SIZES:
                buf[g, pos : pos + n * FREE] = xt[a : a + n].reshape(-1)
                pos += n * FREE + 1
                a += n
        shards.append(buf)
    return shards


def run(x, weight, bias, trace=False):
    """Returns (out [B,C,H,W] f32, exec_time_ns or None)."""
    import concourse.bass_utils as bu
    from concourse.bass_utils import run_bass_kernel_spmd

    if trace:
        _ensure_ntff_hook()
        # zero-egress container: don't try to copy trace artifacts to a bucket
        bu.upload_artifacts = lambda tmpdir: tmpdir

    if "nc" not in _CACHE:
        _CACHE["nc"] = _build()
    nc = _CACHE["nc"]

    wt, bias_sb, mask = _prep_weights(weight, bias)
    shards = _pack_x(x)
    in_maps = [
        {"x": shards[i], "wt": wt, "bias": bias_sb, "mask": mask}
        for i in range(N_CORES)
    ]
    res = run_bass_kernel_spmd(
        nc, in_maps, core_ids=list(range(N_CORES)), trace=trace
    )
    outs = []
    for i in range(N_CORES):
        o = np.asarray(res.results[i]["out"]).reshape(G, C, BG, HW)
        outs.append(o.transpose(0, 2, 1, 3).reshape(SB, C, HW))
    out = np.concatenate(outs, axis=0).astype(np.float32)
    return out.reshape(B, C, H, W), res.exec_time_ns


def kernel(x, weight, bias):
    out, _ = run(x, weight, bias, trace=False)
    return out
